# revision 3
# baseline (speedup 1.0000x reference)
"""Trainium2 Bass kernel for nn_AppearanceComposability (raw bass, manual sems).

Computation (per batch b, channel c, depth d):
    out[b,c,u,v,d] = (sum_{i=u..u+25, j=v..v+25} key[b,c,i,j,d]) * query[b,c,16,16,d]
with B=8, C=64, H=W=32, D=64, K=7 (window L=26). One batch per NeuronCore.

Per-core plan (all fp32):
  SBUF layout X[(c4,i)=128 partitions, (t, j, d)] with channel c = 4*t + c4;
  every partition row is 8KB-contiguous DRAM -> full-rate HWDGE DMA.
  Pass A over j: col[0] = sum_{j<26} x_j  (PE identity-matmul psum-accumulate,
  or DVE/Pool adds, per-chunk choice); col[v] = col[v-1] - x[v-1] + x[v+25].
  Query multiply (i-independent) before pass B. Pass B over i: one fp32 matmul
  per 4-channel tile with banded block-diag stationary [128, 28] ->
  psum[(c4,u), (v,d)]. ScalarE evacuates PSUM; ScalarE issues output DMA.

Raw bass with manual cumulative semaphores; every instruction carries at most
one sem wait (this toolchain's walrus rejects multi-wait instructions).
"""

import math
from contextlib import ExitStack

import numpy as np

try:
    import concourse.bass as bass
except ImportError:
    import sys

    sys.path.insert(0, "/opt/trn_rl_repo")
    import concourse.bass as bass

from concourse import mybir
from concourse.bass_utils import run_bass_kernel_spmd

f32 = mybir.dt.float32

B, C, H, W, D = 8, 64, 32, 32, 64
K = 7
L = H - K + 1  # 26
NT = C // 4  # 16 four-channel tiles
P = 128

# --- tunables ---------------------------------------------------------------
CHUNKS = [6, 5, 4, 1]  # tiles per chunk (sums to 16)
COL0 = ["pe", "dve", "pool", "dve"]  # engine computing col0 per chunk
UPD = ["dve", "dve", "dve", "dve"]  # engine for the 12 update ops per chunk
MULT = ["pool", "pool", "pool", "pool"]  # engine for the 7 query-mult ops
# ----------------------------------------------------------------------------


def build(chunks=None, col0=None, upd=None, mult=None):
    chunks = list(CHUNKS if chunks is None else chunks)
    col0 = list(COL0 if col0 is None else col0)
    upd = list(UPD if upd is None else upd)
    mult = list(MULT if mult is None else mult)
    NCH = len(chunks)
    assert sum(chunks) == NT
    t0s = [sum(chunks[:i]) for i in range(NCH)]
    use_pe_col0 = any(e == "pe" for e in col0)

    nc = bass.Bass()
    x = nc.declare_dram_parameter("x", [C, H, W, D], f32, isOutput=False)
    q2 = nc.declare_dram_parameter("q2", [P, NT, D], f32, isOutput=False)
    a4 = nc.declare_dram_parameter("a4", [P, 4 * K], f32, isOutput=False)
    ident = nc.declare_dram_parameter("ident", [P, P], f32, isOutput=False)
    out = nc.declare_dram_parameter("out", [C, K, K, D], f32, isOutput=True)

    # [128=(c4,i), 16=t, 2048=(j,d)] ; [28=(c4,u), 16=t, 448=(v,d)]
    x_r = x[:].rearrange("(t cf) h w d -> (cf h) t (w d)", cf=4)
    out_r = out[:].rearrange("(t cf) u v d -> (cf u) t (v d)", cf=4)

    ctx = ExitStack()
    with ctx:
        xs = [
            ctx.enter_context(nc.sbuf_tensor(f"xc{i}", [P, tpc, W * D], f32))
            for i, tpc in enumerate(chunks)
        ]
        tas = [
            ctx.enter_context(nc.sbuf_tensor(f"ta{i}", [P, tpc, K, D], f32))
            for i, tpc in enumerate(chunks)
        ]
        obs = [
            ctx.enter_context(nc.sbuf_tensor(f"ob{i}", [4 * K, tpc, K, D], f32))
            for i, tpc in enumerate(chunks)
        ]
        q2_sb = ctx.enter_context(nc.sbuf_tensor("q2sb", [P, NT, D], f32))
        a4_sb = ctx.enter_context(nc.sbuf_tensor("a4sb", [P, 4 * K], f32))
        id_sb = ctx.enter_context(nc.sbuf_tensor("idsb", [P, P], f32))
        pcs = {
            ci: ctx.enter_context(
                nc.psum_tensor(f"pc{ci}", [P, chunks[ci] * D], f32)
            )
            for ci in range(NCH)
            if col0[ci] == "pe"
        }
        pos = [
            ctx.enter_context(nc.psum_tensor(f"po{i}", [4 * K, K * D], f32))
            for i in range(2)
        ]

        dsem = ctx.enter_context(nc.semaphore("dsem"))
        psem = ctx.enter_context(nc.semaphore("psem"))
        vsem = ctx.enter_context(nc.semaphore("vsem"))
        gsem = ctx.enter_context(nc.semaphore("gsem"))
        ssem = ctx.enter_context(nc.semaphore("ssem"))
        osem = ctx.enter_context(nc.semaphore("osem"))

        # ---- static schedule bookkeeping (all cumulative sem targets) ----
        # load order on sync engine:
        loads = ["a4"] + (["ident"] if use_pe_col0 else []) + ["q2"] + [
            f"x{ci}" for ci in range(NCH)
        ]
        d_after = {name: 16 * (i + 1) for i, name in enumerate(loads)}

        # psem: PE program = col0 groups for pe-chunks (chunk order), then all
        # pass-B matmuls (chunk-major, tile order).
        pe_cnt = 0
        psem_col0 = {}
        for ci in range(NCH):
            if col0[ci] == "pe":
                pe_cnt += 1
                psem_col0[ci] = pe_cnt
        psem_passb = {}
        for ci in range(NCH):
            for tt in range(chunks[ci]):
                pe_cnt += 1
                psem_passb[(ci, tt)] = pe_cnt

        # ssem: ACT program = pc copies (pe-chunks, chunk order), then po
        # copies in pass-B order.
        s_cnt = 0
        ssem_pc = {}
        for ci in range(NCH):
            if col0[ci] == "pe":
                s_cnt += 1
                ssem_pc[ci] = s_cnt
        ssem_po = {}
        kglob = 0
        for ci in range(NCH):
            for tt in range(chunks[ci]):
                s_cnt += 1
                ssem_po[kglob] = s_cnt
                kglob += 1
        n_passb = kglob

        # vsem: +1 after each chunk's updates (DVE) or col0-dve stage; track
        # per-chunk value after updates complete.
        v_cnt = 0
        vsem_upd = {}
        vsem_col0 = {}
        for ci in range(NCH):
            if col0[ci] == "dve":
                v_cnt += 1
                vsem_col0[ci] = v_cnt
            if upd[ci] == "dve":
                v_cnt += 1
                vsem_upd[ci] = v_cnt
            if mult[ci] == "dve":
                v_cnt += 1  # mult stage on DVE bumps vsem too
                vsem_upd[(ci, "mult")] = v_cnt

        # gsem: Pool program: per chunk: optional col0, optional updates,
        # optional mult; one inc after each stage it owns.
        g_cnt = 0
        gsem_col0 = {}
        gsem_upd = {}
        gsem_mult = {}
        for ci in range(NCH):
            if col0[ci] == "pool":
                g_cnt += 1
                gsem_col0[ci] = g_cnt
            if upd[ci] == "pool":
                g_cnt += 1
                gsem_upd[ci] = g_cnt
            if mult[ci] == "pool":
                g_cnt += 1
                gsem_mult[ci] = g_cnt

        def stage_done_wait(engine_name, ci, stage):
            """(sem, value) that signals `stage` of chunk ci is complete."""
            eng = {"col0": col0, "upd": upd, "mult": mult}[stage][ci]
            if stage == "col0":
                if eng == "pe":
                    # psum only; consumers of ta[0] must instead wait on the
                    # ACT pc-copy (ssem).
                    return (ssem, ssem_pc[ci])
                if eng == "dve":
                    return (vsem, vsem_col0[ci])
                return (gsem, gsem_col0[ci])
            if stage == "upd":
                if eng == "dve":
                    return (vsem, vsem_upd[ci])
                return (gsem, gsem_upd[ci])
            if eng == "dve":
                return (vsem, vsem_upd[(ci, "mult")])
            return (gsem, gsem_mult[ci])

        last_wait = {}

        def wge(engine, ename, sem, val):
            key = (ename, sem.name if hasattr(sem, "name") else id(sem))
            if last_wait.get(key, -1) < val:
                engine.wait_ge(sem, val)
                last_wait[key] = val

        def emit_col0_adds(eng_ns, ci):
            """col0 via 25 adds on vector/gpsimd namespace `eng_ns`."""
            ta, xc, tpc = tas[ci], xs[ci], chunks[ci]
            eng_ns.tensor_add(
                ta[:, :, 0, :], xc[:, :, 0:D], xc[:, :, D : 2 * D]
            )
            for j in range(2, L):
                eng_ns.tensor_add(
                    ta[:, :, 0, :], ta[:, :, 0, :], xc[:, :, j * D : (j + 1) * D]
                )

        def emit_updates(eng_ns, ci):
            ta, xc = tas[ci], xs[ci]
            for v in range(1, K):
                eng_ns.tensor_sub(
                    ta[:, :, v, :], ta[:, :, v - 1, :], xc[:, :, (v - 1) * D : v * D]
                )
                eng_ns.tensor_add(
                    ta[:, :, v, :],
                    ta[:, :, v, :],
                    xc[:, :, (L + v - 1) * D : (L + v) * D],
                )

        def emit_mult(eng_ns, ci):
            ta, tpc, t0 = tas[ci], chunks[ci], t0s[ci]
            for v in range(K):
                eng_ns.tensor_mul(
                    ta[:, :, v, :], ta[:, :, v, :], q2_sb[:, t0 : t0 + tpc, :]
                )

        with nc.Block() as block:

            @block.sync
            def _(sync):
                sync.dma_start(out=a4_sb[:], in_=a4[:]).then_inc(dsem, 16)
                if use_pe_col0:
                    sync.dma_start(out=id_sb[:], in_=ident[:]).then_inc(dsem, 16)
                sync.dma_start(out=q2_sb[:], in_=q2[:]).then_inc(dsem, 16)
                for ci in range(NCH):
                    t0, tpc = t0s[ci], chunks[ci]
                    sync.dma_start(
                        out=xs[ci][:], in_=x_r[:, t0 : t0 + tpc, :]
                    ).then_inc(dsem, 16)

            @block.tensor
            def _(pe):
                for ci in range(NCH):
                    if col0[ci] != "pe":
                        continue
                    wge(pe, "pe", dsem, d_after[f"x{ci}"])
                    xc, pc, tpc = xs[ci], pcs[ci], chunks[ci]
                    for j in range(L):
                        mm = nc.tensor.matmul(
                            pc[:],
                            id_sb[:],
                            xc[:, :, j * D : (j + 1) * D],
                            start=(j == 0),
                            stop=(j == L - 1),
                        )
                    mm.then_inc(psem, 1)
                kg = 0
                for ci in range(NCH):
                    sem, val = stage_done_wait(pe, ci, "mult")
                    wge(pe, "pe", sem, val)
                    for tt in range(chunks[ci]):
                        if kg >= 2:
                            wge(pe, "pe", ssem, ssem_po[kg - 2])
                        nc.tensor.matmul(
                            pos[kg % 2][:],
                            a4_sb[:],
                            tas[ci][:, tt, :, :],
                            start=True,
                            stop=True,
                        ).then_inc(psem, 1)
                        kg += 1

            @block.scalar
            def _(act):
                for ci in range(NCH):
                    if col0[ci] != "pe":
                        continue
                    wge(act, "act", psem, psem_col0[ci])
                    nc.scalar.copy(
                        out=tas[ci][:, :, 0, :], in_=pcs[ci][:]
                    ).then_inc(ssem, 1)
                kg = 0
                for ci in range(NCH):
                    for tt in range(chunks[ci]):
                        wge(act, "act", psem, psem_passb[(ci, tt)])
                        nc.scalar.copy(
                            out=obs[ci][:, tt, :, :], in_=pos[kg % 2][:]
                        ).then_inc(ssem, 1)
                        kg += 1
                    t0, tpc = t0s[ci], chunks[ci]
                    nc.scalar.dma_start(
                        out=out_r[:, t0 : t0 + tpc, :], in_=obs[ci][:]
                    ).then_inc(osem, 16)
                act.wait_ge(osem, 16 * NCH)

            @block.vector
            def _(vec):
                for ci in range(NCH):
                    if col0[ci] == "dve":
                        wge(vec, "vec", dsem, d_after[f"x{ci}"])
                        emit_col0_adds(nc.vector, ci)
                        nc.vector.engine_nop().then_inc(vsem, 1)
                    if upd[ci] == "dve":
                        sem, val = stage_done_wait(vec, ci, "col0")
                        wge(vec, "vec", sem, val)
                        wge(vec, "vec", dsem, d_after[f"x{ci}"])
                        emit_updates(nc.vector, ci)
                        nc.vector.engine_nop().then_inc(vsem, 1)
                    if mult[ci] == "dve":
                        sem, val = stage_done_wait(vec, ci, "upd")
                        wge(vec, "vec", sem, val)
                        emit_mult(nc.vector, ci)
                        nc.vector.engine_nop().then_inc(vsem, 1)

            @block.gpsimd
            def _(gp):
                for ci in range(NCH):
                    if col0[ci] == "pool":
                        wge(gp, "gp", dsem, d_after[f"x{ci}"])
                        emit_col0_adds(nc.gpsimd, ci)
                        nc.gpsimd.engine_nop().then_inc(gsem, 1)
                    if upd[ci] == "pool":
                        sem, val = stage_done_wait(gp, ci, "col0")
                        wge(gp, "gp", sem, val)
                        wge(gp, "gp", dsem, d_after[f"x{ci}"])
                        emit_updates(nc.gpsimd, ci)
                        nc.gpsimd.engine_nop().then_inc(gsem, 1)
                    if mult[ci] == "pool":
                        sem, val = stage_done_wait(gp, ci, "upd")
                        wge(gp, "gp", sem, val)
                        wge(gp, "gp", dsem, d_after["q2"])
                        emit_mult(nc.gpsimd, ci)
                        nc.gpsimd.engine_nop().then_inc(gsem, 1)

    return nc


def _host_inputs(key_map, query_map):
    a4 = np.zeros((P, 4 * K), dtype=np.float32)
    for c4 in range(4):
        for u in range(K):
            a4[c4 * 32 + u : c4 * 32 + u + L, c4 * K + u] = 1.0
    ident = np.eye(P, dtype=np.float32)

    qc = np.asarray(query_map[:, :, H // 2, W // 2, :], dtype=np.float32)  # [B,C,D]
    in_maps = []
    for b in range(B):
        q2 = np.transpose(qc[b].reshape(NT, 4, D), (1, 0, 2))  # [4, NT, D]
        q2 = np.broadcast_to(q2[:, None, :, :], (4, 32, NT, D)).reshape(P, NT, D)
        in_maps.append(
            {
                "x": np.ascontiguousarray(key_map[b], dtype=np.float32),
                "q2": np.ascontiguousarray(q2),
                "a4": a4,
                "ident": ident,
            }
        )
    return in_maps


_cache = {}


def _get_nc():
    key = (tuple(CHUNKS), tuple(COL0), tuple(UPD), tuple(MULT))
    if key not in _cache:
        _cache[key] = build()
    return _cache[key]


def kernel(key_map, query_map, _trace=False):
    nc = _get_nc()
    in_maps = _host_inputs(key_map, query_map)
    res = run_bass_kernel_spmd(nc, in_maps, core_ids=list(range(B)), trace=_trace)
    out = np.stack([res.results[i]["out"] for i in range(B)])
    if _trace:
        return out, res
    return out


# revision 4
# speedup vs baseline: 38.0556x; 38.0556x over previous
"""Trainium2 Bass kernel for nn_AppearanceComposability (raw bass, manual sems).

Computation (per batch b, channel c, depth d):
    out[b,c,u,v,d] = (sum_{i=u..u+25, j=v..v+25} key[b,c,i,j,d]) * query[b,c,16,16,d]
with B=8, C=64, H=W=32, D=64, K=7 (window L=26). One batch per NeuronCore.

Per-core plan (all fp32):
  SBUF layout X[(c4,i)=128 partitions, (t, j, d)] with channel c = 4*t + c4;
  every partition row is 8KB-contiguous DRAM -> full-rate HWDGE DMA.
  Pass A over j: col[0] = sum_{j<26} x_j  (PE identity-matmul psum-accumulate,
  or DVE/Pool adds, per-chunk choice); col[v] = col[v-1] - x[v-1] + x[v+25].
  Query multiply (i-independent) before pass B. Pass B over i: one fp32 matmul
  per 4-channel tile with banded block-diag stationary [128, 28] ->
  psum[(c4,u), (v,d)]. ScalarE evacuates PSUM; ScalarE issues output DMA.

Raw bass with manual cumulative semaphores; every instruction carries at most
one sem wait (this toolchain's walrus rejects multi-wait instructions).

`reps` repeats the whole body inside one NEFF (for differential timing).
"""

from contextlib import ExitStack

import numpy as np

try:
    import concourse.bass as bass
except ImportError:
    import sys

    sys.path.insert(0, "/opt/trn_rl_repo")
    import concourse.bass as bass

from concourse import mybir
from concourse.bass_utils import run_bass_kernel_spmd

f32 = mybir.dt.float32

B, C, H, W, D = 8, 64, 32, 32, 64
K = 7
L = H - K + 1  # 26
NT = C // 4  # 16 four-channel tiles
P = 128

# --- tunables ---------------------------------------------------------------
CHUNKS = [6, 5, 4, 1]  # tiles per chunk (sums to 16)
COL0 = ["pe", "dve", "pool", "dve"]  # engine computing col0 per chunk
UPD = ["dve", "dve", "dve", "dve"]  # engine for the 12 update ops per chunk
MULT = ["pool", "pool", "pool", "pool"]  # engine for the 7 query-mult ops
# ----------------------------------------------------------------------------


def build(chunks=None, col0=None, upd=None, mult=None, reps=1):
    chunks = list(CHUNKS if chunks is None else chunks)
    col0 = list(COL0 if col0 is None else col0)
    upd = list(UPD if upd is None else upd)
    mult = list(MULT if mult is None else mult)
    NCH = len(chunks)
    assert sum(chunks) == NT
    t0s = [sum(chunks[:i]) for i in range(NCH)]
    use_pe_col0 = any(e == "pe" for e in col0)

    nc = bass.Bass()
    x = nc.declare_dram_parameter("x", [C, H, W, D], f32, isOutput=False)
    q2 = nc.declare_dram_parameter("q2", [P, NT, D], f32, isOutput=False)
    a4 = nc.declare_dram_parameter("a4", [P, 4 * K], f32, isOutput=False)
    ident = nc.declare_dram_parameter("ident", [P, P], f32, isOutput=False)
    out = nc.declare_dram_parameter("out", [C, K, K, D], f32, isOutput=True)

    # [128=(c4,i), 16=t, 2048=(j,d)] ; [28=(c4,u), 16=t, 448=(v,d)]
    x_r = x[:].rearrange("(t cf) h w d -> (cf h) t (w d)", cf=4)
    out_r = out[:].rearrange("(t cf) u v d -> (cf u) t (v d)", cf=4)

    ctx = ExitStack()
    with ctx:
        xs = [
            ctx.enter_context(nc.sbuf_tensor(f"xc{i}", [P, tpc, W * D], f32))
            for i, tpc in enumerate(chunks)
        ]
        tas = [
            ctx.enter_context(nc.sbuf_tensor(f"ta{i}", [P, tpc, K, D], f32))
            for i, tpc in enumerate(chunks)
        ]
        obs = [
            ctx.enter_context(nc.sbuf_tensor(f"ob{i}", [4 * K, tpc, K, D], f32))
            for i, tpc in enumerate(chunks)
        ]
        q2_sb = ctx.enter_context(nc.sbuf_tensor("q2sb", [P, NT, D], f32))
        a4_sb = ctx.enter_context(nc.sbuf_tensor("a4sb", [P, 4 * K], f32))
        id_sb = ctx.enter_context(nc.sbuf_tensor("idsb", [P, P], f32))
        pcs = {
            ci: ctx.enter_context(
                nc.psum_tensor(f"pc{ci}", [P, chunks[ci] * D], f32)
            )
            for ci in range(NCH)
            if col0[ci] == "pe"
        }
        pos = [
            ctx.enter_context(nc.psum_tensor(f"po{i}", [4 * K, K * D], f32))
            for i in range(2)
        ]

        dsem = ctx.enter_context(nc.semaphore("dsem"))
        psem = ctx.enter_context(nc.semaphore("psem"))
        vsem = ctx.enter_context(nc.semaphore("vsem"))
        gsem = ctx.enter_context(nc.semaphore("gsem"))
        ssem = ctx.enter_context(nc.semaphore("ssem"))
        osem = ctx.enter_context(nc.semaphore("osem"))

        # ---- static per-rep schedule bookkeeping (cumulative sem targets) ----
        n_consts = 2 + (1 if use_pe_col0 else 0)  # a4, [ident], q2

        def d_x(r, ci):  # dsem value once chunk ci of rep r is loaded
            return 16 * (n_consts + r * NCH + ci + 1)

        # psem: PE program per rep = col0 groups (pe-chunks in order), then all
        # pass-B matmuls (chunk-major).
        pe_cnt = 0
        psem_col0 = {}
        for ci in range(NCH):
            if col0[ci] == "pe":
                pe_cnt += 1
                psem_col0[ci] = pe_cnt
        psem_passb = {}
        for ci in range(NCH):
            for tt in range(chunks[ci]):
                pe_cnt += 1
                psem_passb[(ci, tt)] = pe_cnt
        pe_per = pe_cnt

        # ssem: ACT per rep = pc copies (pe-chunks), then po copies.
        s_cnt = 0
        ssem_pc = {}
        for ci in range(NCH):
            if col0[ci] == "pe":
                s_cnt += 1
                ssem_pc[ci] = s_cnt
        ssem_po = {}
        kglob = 0
        for ci in range(NCH):
            for tt in range(chunks[ci]):
                s_cnt += 1
                ssem_po[kglob] = s_cnt
                kglob += 1
        s_per = s_cnt
        n_passb = kglob

        # vsem / gsem: +1 after each owned stage, chunk-major order.
        v_cnt = 0
        vsem_stage = {}
        g_cnt = 0
        gsem_stage = {}
        for ci in range(NCH):
            if col0[ci] == "dve":
                v_cnt += 1
                vsem_stage[(ci, "col0")] = v_cnt
            if col0[ci] == "pool":
                g_cnt += 1
                gsem_stage[(ci, "col0")] = g_cnt
            if upd[ci] == "dve":
                v_cnt += 1
                vsem_stage[(ci, "upd")] = v_cnt
            if upd[ci] == "pool":
                g_cnt += 1
                gsem_stage[(ci, "upd")] = g_cnt
            if mult[ci] == "dve":
                v_cnt += 1
                vsem_stage[(ci, "mult")] = v_cnt
            if mult[ci] == "pool":
                g_cnt += 1
                gsem_stage[(ci, "mult")] = g_cnt
        v_per = v_cnt
        g_per = g_cnt

        def stage_done_wait(r, ci, stage):
            """(sem, cumulative value) signalling `stage` of chunk ci done."""
            eng = {"col0": col0, "upd": upd, "mult": mult}[stage][ci]
            if stage == "col0" and eng == "pe":
                return (ssem, r * s_per + ssem_pc[ci])
            if eng == "dve":
                return (vsem, r * v_per + vsem_stage[(ci, stage)])
            return (gsem, r * g_per + gsem_stage[(ci, stage)])

        last_wait = {}

        def wge(engine, ename, sem, val):
            key = (ename, id(sem))
            if last_wait.get(key, -1) < val:
                engine.wait_ge(sem, val)
                last_wait[key] = val

        def emit_col0_adds(eng_ns, ci):
            ta, xc = tas[ci], xs[ci]
            eng_ns.tensor_add(ta[:, :, 0, :], xc[:, :, 0:D], xc[:, :, D : 2 * D])
            for j in range(2, L):
                eng_ns.tensor_add(
                    ta[:, :, 0, :], ta[:, :, 0, :], xc[:, :, j * D : (j + 1) * D]
                )

        def emit_updates(eng_ns, ci):
            ta, xc = tas[ci], xs[ci]
            for v in range(1, K):
                eng_ns.tensor_sub(
                    ta[:, :, v, :], ta[:, :, v - 1, :], xc[:, :, (v - 1) * D : v * D]
                )
                eng_ns.tensor_add(
                    ta[:, :, v, :],
                    ta[:, :, v, :],
                    xc[:, :, (L + v - 1) * D : (L + v) * D],
                )

        def emit_mult(eng_ns, ci):
            ta, tpc, t0 = tas[ci], chunks[ci], t0s[ci]
            for v in range(K):
                eng_ns.tensor_mul(
                    ta[:, :, v, :], ta[:, :, v, :], q2_sb[:, t0 : t0 + tpc, :]
                )

        with nc.Block() as block:

            @block.sync
            def _(sync):
                sync.dma_start(out=a4_sb[:], in_=a4[:]).then_inc(dsem, 16)
                if use_pe_col0:
                    sync.dma_start(out=id_sb[:], in_=ident[:]).then_inc(dsem, 16)
                sync.dma_start(out=q2_sb[:], in_=q2[:]).then_inc(dsem, 16)
                for r in range(reps):
                    if r:
                        sync.wait_ge(osem, 16 * NCH * r)
                    for ci in range(NCH):
                        t0, tpc = t0s[ci], chunks[ci]
                        sync.dma_start(
                            out=xs[ci][:], in_=x_r[:, t0 : t0 + tpc, :]
                        ).then_inc(dsem, 16)

            @block.tensor
            def _(pe):
                for r in range(reps):
                    for ci in range(NCH):
                        if col0[ci] != "pe":
                            continue
                        wge(pe, "pe", dsem, d_x(r, ci))
                        xc, pc = xs[ci], pcs[ci]
                        for j in range(L):
                            mm = nc.tensor.matmul(
                                pc[:],
                                id_sb[:],
                                xc[:, :, j * D : (j + 1) * D],
                                start=(j == 0),
                                stop=(j == L - 1),
                            )
                        mm.then_inc(psem, 1)
                    kg = 0
                    for ci in range(NCH):
                        sem, val = stage_done_wait(r, ci, "mult")
                        wge(pe, "pe", sem, val)
                        for tt in range(chunks[ci]):
                            kabs = r * n_passb + kg
                            if kabs >= 2:
                                # WAR on po slot: its previous reader (ACT copy
                                # kabs-2) must be done.
                                prev = kabs - 2
                                pr, pk = divmod(prev, n_passb)
                                wge(pe, "pe", ssem, pr * s_per + ssem_po[pk])
                            nc.tensor.matmul(
                                pos[kabs % 2][:],
                                a4_sb[:],
                                tas[ci][:, tt, :, :],
                                start=True,
                                stop=True,
                            ).then_inc(psem, 1)
                            kg += 1

            @block.scalar
            def _(act):
                for r in range(reps):
                    for ci in range(NCH):
                        if col0[ci] != "pe":
                            continue
                        wge(act, "act", psem, r * pe_per + psem_col0[ci])
                        nc.scalar.copy(
                            out=tas[ci][:, :, 0, :], in_=pcs[ci][:]
                        ).then_inc(ssem, 1)
                    kg = 0
                    for ci in range(NCH):
                        for tt in range(chunks[ci]):
                            wge(act, "act", psem, r * pe_per + psem_passb[(ci, tt)])
                            nc.scalar.copy(
                                out=obs[ci][:, tt, :, :],
                                in_=pos[(r * n_passb + kg) % 2][:],
                            ).then_inc(ssem, 1)
                            kg += 1
                        t0, tpc = t0s[ci], chunks[ci]
                        nc.scalar.dma_start(
                            out=out_r[:, t0 : t0 + tpc, :], in_=obs[ci][:]
                        ).then_inc(osem, 16)
                act.wait_ge(osem, 16 * NCH * reps)

            @block.vector
            def _(vec):
                for r in range(reps):
                    for ci in range(NCH):
                        if col0[ci] == "dve":
                            wge(vec, "vec", dsem, d_x(r, ci))
                            emit_col0_adds(nc.vector, ci)
                            nc.vector.engine_nop().then_inc(vsem, 1)
                        if upd[ci] == "dve":
                            sem, val = stage_done_wait(r, ci, "col0")
                            wge(vec, "vec", sem, val)
                            wge(vec, "vec", dsem, d_x(r, ci))
                            emit_updates(nc.vector, ci)
                            nc.vector.engine_nop().then_inc(vsem, 1)
                        if mult[ci] == "dve":
                            sem, val = stage_done_wait(r, ci, "upd")
                            wge(vec, "vec", sem, val)
                            emit_mult(nc.vector, ci)
                            nc.vector.engine_nop().then_inc(vsem, 1)

            @block.gpsimd
            def _(gp):
                for r in range(reps):
                    for ci in range(NCH):
                        if col0[ci] == "pool":
                            wge(gp, "gp", dsem, d_x(r, ci))
                            emit_col0_adds(nc.gpsimd, ci)
                            nc.gpsimd.engine_nop().then_inc(gsem, 1)
                        if upd[ci] == "pool":
                            sem, val = stage_done_wait(r, ci, "col0")
                            wge(gp, "gp", sem, val)
                            wge(gp, "gp", dsem, d_x(r, ci))
                            emit_updates(nc.gpsimd, ci)
                            nc.gpsimd.engine_nop().then_inc(gsem, 1)
                        if mult[ci] == "pool":
                            sem, val = stage_done_wait(r, ci, "upd")
                            wge(gp, "gp", sem, val)
                            wge(gp, "gp", dsem, 16 * n_consts)
                            emit_mult(nc.gpsimd, ci)
                            nc.gpsimd.engine_nop().then_inc(gsem, 1)

    return nc


def _host_inputs(key_map, query_map):
    a4 = np.zeros((P, 4 * K), dtype=np.float32)
    for c4 in range(4):
        for u in range(K):
            a4[c4 * 32 + u : c4 * 32 + u + L, c4 * K + u] = 1.0
    ident = np.eye(P, dtype=np.float32)

    qc = np.asarray(query_map[:, :, H // 2, W // 2, :], dtype=np.float32)  # [B,C,D]
    in_maps = []
    for b in range(B):
        q2 = np.transpose(qc[b].reshape(NT, 4, D), (1, 0, 2))  # [4, NT, D]
        q2 = np.broadcast_to(q2[:, None, :, :], (4, 32, NT, D)).reshape(P, NT, D)
        in_maps.append(
            {
                "x": np.ascontiguousarray(key_map[b], dtype=np.float32),
                "q2": np.ascontiguousarray(q2),
                "a4": a4,
                "ident": ident,
            }
        )
    return in_maps


_cache = {}


def _get_nc(reps=1):
    key = (tuple(CHUNKS), tuple(COL0), tuple(UPD), tuple(MULT), reps)
    if key not in _cache:
        _cache[key] = build(reps=reps)
    return _cache[key]


def kernel(key_map, query_map, _trace=False):
    nc = _get_nc()
    in_maps = _host_inputs(key_map, query_map)
    res = run_bass_kernel_spmd(nc, in_maps, core_ids=list(range(B)), trace=_trace)
    out = np.stack([res.results[i]["out"] for i in range(B)])
    if _trace:
        return out, res
    return out


# revision 5
# speedup vs baseline: 78.8668x; 2.0724x over previous
"""Trainium2 Bass kernel for nn_AppearanceComposability (raw bass, manual sems).

Computation (per batch b, channel c, depth d):
    out[b,c,u,v,d] = (sum_{i=u..u+25, j=v..j+25} key[b,c,i,j,d]) * query[b,c,16,16,d]
with B=8, C=64, H=W=32, D=64, K=7 (window L=26). One batch per NeuronCore.

Per-core plan:
  Host pre-arranges x to the SBUF layout [(c4,i)=128 partitions, (t, j, d)]
  (channel c = 4*t + c4) and casts to bf16 -> contiguous full-rate DMA at half
  the f32 traffic. Output gate is rel_err < 2e-2; bf16 inputs give ~4e-3.

  Pass A over j: col[0] = sum_{j<26} x_j via 26 accumulating identity matmuls
  (TensorE, fp32 PSUM accumulation - exact) or DVE/Pool adds per chunk;
  col[v] = col[v-1] - x[v-1] + x[v+25] (12 DVE ops/chunk, bf16 2x mode).
  Query multiply (i-independent, commutes with pass B): GpSimd.
  Pass B over i: one bf16 matmul per 4-channel tile with banded block-diag
  stationary [128, 28] -> psum[(c4,u), (v,d)] f32. ScalarE evacuates PSUM and
  issues output DMA (f32).

Raw bass with manual cumulative semaphores; every instruction carries at most
one sem wait (this toolchain's walrus rejects multi-wait instructions).

`reps` repeats the whole body inside one NEFF (differential timing).
"""

from contextlib import ExitStack

import numpy as np

try:
    import concourse.bass as bass
except ImportError:
    import sys

    sys.path.insert(0, "/opt/trn_rl_repo")
    import concourse.bass as bass

from concourse import mybir
from concourse.bass_utils import run_bass_kernel_spmd

f32 = mybir.dt.float32
bf16 = mybir.dt.bfloat16

B, C, H, W, D = 8, 64, 32, 32, 64
K = 7
L = H - K + 1  # 26
NT = C // 4  # 16 four-channel tiles
P = 128

# --- tunables ---------------------------------------------------------------
DT = "bf16"  # "bf16" | "f32" compute/storage for x, ta, q2, matmul operands
CHUNKS = [5, 5, 3, 3]  # tiles per chunk (sums to 16)
COL0 = ["pe", "pe", "pe", "pe"]  # engine computing col0 per chunk
UPD = ["dve", "dve", "dve", "dve"]  # engine for the 12 update ops per chunk
MULT = ["pool", "pool", "pool", "pool"]  # engine for the 7 query-mult ops
# ----------------------------------------------------------------------------


def build(chunks=None, col0=None, upd=None, mult=None, reps=1, dt=None):
    chunks = list(CHUNKS if chunks is None else chunks)
    col0 = list(COL0 if col0 is None else col0)
    upd = list(UPD if upd is None else upd)
    mult = list(MULT if mult is None else mult)
    cdt = {"bf16": bf16, "f32": f32}[DT if dt is None else dt]
    NCH = len(chunks)
    assert sum(chunks) == NT
    t0s = [sum(chunks[:i]) for i in range(NCH)]
    use_pe_col0 = any(e == "pe" for e in col0)

    nc = bass.Bass()
    # x is host-pretransposed to [(c4,i), t, (j,d)] and cast to cdt.
    x = nc.declare_dram_parameter("x", [P, NT, W * D], cdt, isOutput=False)
    q2 = nc.declare_dram_parameter("q2", [P, NT, D], cdt, isOutput=False)
    a4 = nc.declare_dram_parameter("a4", [P, 4 * K], cdt, isOutput=False)
    ident = nc.declare_dram_parameter("ident", [P, P], cdt, isOutput=False)
    out = nc.declare_dram_parameter("out", [C, K, K, D], f32, isOutput=True)

    # [28=(c4,u), 16=t, 448=(v,d)]
    out_r = out[:].rearrange("(t cf) u v d -> (cf u) t (v d)", cf=4)

    ctx = ExitStack()
    with ctx:
        xs = [
            ctx.enter_context(nc.sbuf_tensor(f"xc{i}", [P, tpc, W * D], cdt))
            for i, tpc in enumerate(chunks)
        ]
        tas = [
            ctx.enter_context(nc.sbuf_tensor(f"ta{i}", [P, tpc, K, D], cdt))
            for i, tpc in enumerate(chunks)
        ]
        obs = [
            ctx.enter_context(nc.sbuf_tensor(f"ob{i}", [4 * K, tpc, K, D], f32))
            for i, tpc in enumerate(chunks)
        ]
        q2_sb = ctx.enter_context(nc.sbuf_tensor("q2sb", [P, NT, D], cdt))
        a4_sb = ctx.enter_context(nc.sbuf_tensor("a4sb", [P, 4 * K], cdt))
        id_sb = ctx.enter_context(nc.sbuf_tensor("idsb", [P, P], cdt))
        pcs = {
            ci: ctx.enter_context(
                nc.psum_tensor(f"pc{ci}", [P, chunks[ci] * D], f32)
            )
            for ci in range(NCH)
            if col0[ci] == "pe"
        }
        pos = [
            ctx.enter_context(nc.psum_tensor(f"po{i}", [4 * K, K * D], f32))
            for i in range(2)
        ]

        dsem = ctx.enter_context(nc.semaphore("dsem"))
        psem = ctx.enter_context(nc.semaphore("psem"))
        vsem = ctx.enter_context(nc.semaphore("vsem"))
        gsem = ctx.enter_context(nc.semaphore("gsem"))
        ssem = ctx.enter_context(nc.semaphore("ssem"))
        osem = ctx.enter_context(nc.semaphore("osem"))

        # ---- static per-rep schedule bookkeeping (cumulative sem targets) ----
        # load order: x0 first (compute starts earliest), then consts, then
        # the remaining chunks.
        loads = ["x0"] + (["ident"] if use_pe_col0 else []) + ["q2", "a4"] + [
            f"x{ci}" for ci in range(1, NCH)
        ]
        n_consts = len(loads) - NCH
        base_idx = {name: i for i, name in enumerate(loads)}

        def d_x(r, ci):  # dsem value once chunk ci of rep r is loaded
            if r == 0:
                return 16 * (base_idx[f"x{ci}"] + 1)
            return 16 * (len(loads) + (r - 1) * NCH + ci + 1)

        d_consts = 16 * len(loads)  # all rep-0 loads (incl consts) done

        # psem: PE program per rep = col0 groups (pe-chunks in order), then all
        # pass-B matmuls (chunk-major).
        pe_cnt = 0
        psem_col0 = {}
        for ci in range(NCH):
            if col0[ci] == "pe":
                pe_cnt += 1
                psem_col0[ci] = pe_cnt
        psem_passb = {}
        for ci in range(NCH):
            for tt in range(chunks[ci]):
                pe_cnt += 1
                psem_passb[(ci, tt)] = pe_cnt
        pe_per = pe_cnt

        # ssem: ACT per rep = pc copies (pe-chunks), then po copies.
        s_cnt = 0
        ssem_pc = {}
        for ci in range(NCH):
            if col0[ci] == "pe":
                s_cnt += 1
                ssem_pc[ci] = s_cnt
        ssem_po = {}
        kglob = 0
        for ci in range(NCH):
            for tt in range(chunks[ci]):
                s_cnt += 1
                ssem_po[kglob] = s_cnt
                kglob += 1
        s_per = s_cnt
        n_passb = kglob

        # vsem / gsem: +1 after each owned stage, chunk-major order.
        v_cnt = 0
        vsem_stage = {}
        g_cnt = 0
        gsem_stage = {}
        for ci in range(NCH):
            if col0[ci] == "dve":
                v_cnt += 1
                vsem_stage[(ci, "col0")] = v_cnt
            if col0[ci] == "pool":
                g_cnt += 1
                gsem_stage[(ci, "col0")] = g_cnt
            if upd[ci] == "dve":
                v_cnt += 1
                vsem_stage[(ci, "upd")] = v_cnt
            if upd[ci] == "pool":
                g_cnt += 1
                gsem_stage[(ci, "upd")] = g_cnt
            if mult[ci] == "dve":
                v_cnt += 1
                vsem_stage[(ci, "mult")] = v_cnt
            if mult[ci] == "pool":
                g_cnt += 1
                gsem_stage[(ci, "mult")] = g_cnt
        v_per = v_cnt
        g_per = g_cnt

        def stage_done_wait(r, ci, stage):
            """(sem, cumulative value) signalling `stage` of chunk ci done."""
            eng = {"col0": col0, "upd": upd, "mult": mult}[stage][ci]
            if stage == "col0" and eng == "pe":
                return (ssem, r * s_per + ssem_pc[ci])
            if eng == "dve":
                return (vsem, r * v_per + vsem_stage[(ci, stage)])
            return (gsem, r * g_per + gsem_stage[(ci, stage)])

        last_wait = {}

        def wge(engine, ename, sem, val):
            key = (ename, id(sem))
            if last_wait.get(key, -1) < val:
                engine.wait_ge(sem, val)
                last_wait[key] = val

        def emit_col0_adds(eng_ns, ci):
            ta, xc = tas[ci], xs[ci]
            eng_ns.tensor_add(ta[:, :, 0, :], xc[:, :, 0:D], xc[:, :, D : 2 * D])
            for j in range(2, L):
                eng_ns.tensor_add(
                    ta[:, :, 0, :], ta[:, :, 0, :], xc[:, :, j * D : (j + 1) * D]
                )

        def emit_updates(eng_ns, ci):
            ta, xc = tas[ci], xs[ci]
            for v in range(1, K):
                eng_ns.tensor_sub(
                    ta[:, :, v, :], ta[:, :, v - 1, :], xc[:, :, (v - 1) * D : v * D]
                )
                eng_ns.tensor_add(
                    ta[:, :, v, :],
                    ta[:, :, v, :],
                    xc[:, :, (L + v - 1) * D : (L + v) * D],
                )

        def emit_mult(eng_ns, ci):
            ta, tpc, t0 = tas[ci], chunks[ci], t0s[ci]
            for v in range(K):
                eng_ns.tensor_mul(
                    ta[:, :, v, :], ta[:, :, v, :], q2_sb[:, t0 : t0 + tpc, :]
                )

        with nc.Block() as block:

            @block.sync
            def _(sync):
                def load(name):
                    if name == "ident":
                        sync.dma_start(out=id_sb[:], in_=ident[:]).then_inc(dsem, 16)
                    elif name == "q2":
                        sync.dma_start(out=q2_sb[:], in_=q2[:]).then_inc(dsem, 16)
                    elif name == "a4":
                        sync.dma_start(out=a4_sb[:], in_=a4[:]).then_inc(dsem, 16)
                    else:
                        ci = int(name[1:])
                        t0, tpc = t0s[ci], chunks[ci]
                        sync.dma_start(
                            out=xs[ci][:], in_=x[:, t0 : t0 + tpc, :]
                        ).then_inc(dsem, 16)

                for name in loads:
                    load(name)
                for r in range(1, reps):
                    sync.wait_ge(osem, 16 * NCH * r)
                    for ci in range(NCH):
                        load(f"x{ci}")

            @block.tensor
            def _(pe):
                for r in range(reps):
                    for ci in range(NCH):
                        if col0[ci] != "pe":
                            continue
                        wge(pe, "pe", dsem, d_x(r, ci))
                        if ci == 0 and r == 0:
                            wge(pe, "pe", dsem, 16 * (base_idx["ident"] + 1))
                        xc, pc = xs[ci], pcs[ci]
                        for j in range(L):
                            mm = nc.tensor.matmul(
                                pc[:],
                                id_sb[:],
                                xc[:, :, j * D : (j + 1) * D],
                                start=(j == 0),
                                stop=(j == L - 1),
                            )
                        mm.then_inc(psem, 1)
                    kg = 0
                    for ci in range(NCH):
                        sem, val = stage_done_wait(r, ci, "mult")
                        wge(pe, "pe", sem, val)
                        if r == 0 and ci == 0:
                            wge(pe, "pe", dsem, 16 * (base_idx["a4"] + 1))
                        for tt in range(chunks[ci]):
                            kabs = r * n_passb + kg
                            if kabs >= 2:
                                prev = kabs - 2
                                pr, pk = divmod(prev, n_passb)
                                wge(pe, "pe", ssem, pr * s_per + ssem_po[pk])
                            nc.tensor.matmul(
                                pos[kabs % 2][:],
                                a4_sb[:],
                                tas[ci][:, tt, :, :],
                                start=True,
                                stop=True,
                            ).then_inc(psem, 1)
                            kg += 1

            @block.scalar
            def _(act):
                for r in range(reps):
                    for ci in range(NCH):
                        if col0[ci] != "pe":
                            continue
                        wge(act, "act", psem, r * pe_per + psem_col0[ci])
                        nc.scalar.copy(
                            out=tas[ci][:, :, 0, :], in_=pcs[ci][:]
                        ).then_inc(ssem, 1)
                    kg = 0
                    for ci in range(NCH):
                        for tt in range(chunks[ci]):
                            wge(act, "act", psem, r * pe_per + psem_passb[(ci, tt)])
                            nc.scalar.copy(
                                out=obs[ci][:, tt, :, :],
                                in_=pos[(r * n_passb + kg) % 2][:],
                            ).then_inc(ssem, 1)
                            kg += 1
                        t0, tpc = t0s[ci], chunks[ci]
                        nc.scalar.dma_start(
                            out=out_r[:, t0 : t0 + tpc, :], in_=obs[ci][:]
                        ).then_inc(osem, 16)
                act.wait_ge(osem, 16 * NCH * reps)

            @block.vector
            def _(vec):
                for r in range(reps):
                    for ci in range(NCH):
                        if col0[ci] == "dve":
                            wge(vec, "vec", dsem, d_x(r, ci))
                            emit_col0_adds(nc.vector, ci)
                            nc.vector.engine_nop().then_inc(vsem, 1)
                        if upd[ci] == "dve":
                            sem, val = stage_done_wait(r, ci, "col0")
                            wge(vec, "vec", sem, val)
                            wge(vec, "vec", dsem, d_x(r, ci))
                            emit_updates(nc.vector, ci)
                            nc.vector.engine_nop().then_inc(vsem, 1)
                        if mult[ci] == "dve":
                            sem, val = stage_done_wait(r, ci, "upd")
                            wge(vec, "vec", sem, val)
                            wge(vec, "vec", dsem, 16 * (base_idx["q2"] + 1))
                            emit_mult(nc.vector, ci)
                            nc.vector.engine_nop().then_inc(vsem, 1)

            @block.gpsimd
            def _(gp):
                for r in range(reps):
                    for ci in range(NCH):
                        if col0[ci] == "pool":
                            wge(gp, "gp", dsem, d_x(r, ci))
                            emit_col0_adds(nc.gpsimd, ci)
                            nc.gpsimd.engine_nop().then_inc(gsem, 1)
                        if upd[ci] == "pool":
                            sem, val = stage_done_wait(r, ci, "col0")
                            wge(gp, "gp", sem, val)
                            wge(gp, "gp", dsem, d_x(r, ci))
                            emit_updates(nc.gpsimd, ci)
                            nc.gpsimd.engine_nop().then_inc(gsem, 1)
                        if mult[ci] == "pool":
                            sem, val = stage_done_wait(r, ci, "upd")
                            wge(gp, "gp", sem, val)
                            wge(gp, "gp", dsem, 16 * (base_idx["q2"] + 1))
                            emit_mult(nc.gpsimd, ci)
                            nc.gpsimd.engine_nop().then_inc(gsem, 1)

    return nc


def _host_inputs(key_map, query_map, dt=None):
    np_dt = np.float32 if (DT if dt is None else dt) == "f32" else mybir.dt.np(bf16)
    a4 = np.zeros((P, 4 * K), dtype=np.float32)
    for c4 in range(4):
        for u in range(K):
            a4[c4 * 32 + u : c4 * 32 + u + L, c4 * K + u] = 1.0
    a4 = a4.astype(np_dt)
    ident = np.eye(P, dtype=np.float32).astype(np_dt)

    key_map = np.asarray(key_map, dtype=np.float32)
    qc = np.asarray(query_map[:, :, H // 2, W // 2, :], dtype=np.float32)  # [B,C,D]
    in_maps = []
    for b in range(B):
        # x_host[(c4,i), t, (j,d)] = key[b, 4t+c4, i, j, d]
        xb = (
            key_map[b]
            .reshape(NT, 4, H, W * D)
            .transpose(1, 2, 0, 3)
            .reshape(P, NT, W * D)
            .astype(np_dt)
        )
        q2 = np.transpose(qc[b].reshape(NT, 4, D), (1, 0, 2))  # [4, NT, D]
        q2 = (
            np.broadcast_to(q2[:, None, :, :], (4, 32, NT, D))
            .reshape(P, NT, D)
            .astype(np_dt)
        )
        in_maps.append(
            {
                "x": np.ascontiguousarray(xb),
                "q2": np.ascontiguousarray(q2),
                "a4": a4,
                "ident": ident,
            }
        )
    return in_maps


_cache = {}


def _get_nc(reps=1):
    key = (tuple(CHUNKS), tuple(COL0), tuple(UPD), tuple(MULT), reps, DT)
    if key not in _cache:
        _cache[key] = build(reps=reps)
    return _cache[key]


def kernel(key_map, query_map, _trace=False):
    nc = _get_nc()
    in_maps = _host_inputs(key_map, query_map)
    res = run_bass_kernel_spmd(nc, in_maps, core_ids=list(range(B)), trace=_trace)
    out = np.stack([res.results[i]["out"] for i in range(B)])
    if _trace:
        return out, res
    return out
